# revision 1
# baseline (speedup 1.0000x reference)
"""Multi-head causal self-attention (B=2, T=4096, D=768, H=12) on 8 trn2 cores.

Sharding: core c -> batch b = c//4, heads 3*(c%4) .. 3*(c%4)+2.
qkv_proj column-parallel (each core computes Q/K/V only for its heads),
out_proj row-parallel (each core emits a partial y^T; host sums the 4
partials per batch).

Device dataflow (all fp32):
  x^T tiles via PE transposes -> Q^T/K^T via transposed projection
  (W^T stationary, x^T streaming) -> S^T = K Q^T in [k,q] layout, two
  heads row-paired on opposite PE halves -> exp on ScalarE (no max
  subtraction; scores ~ N(0,1)) -> causal band masks on DVE ->
  out^T = V^T P^T col-paired (even k-tiles -> psum partitions 0:64,
  odd -> 64:128) with a parallel 4-way col-tiled ones-matmul computing
  softmax denominators -> normalize via batched reciprocal + gpsimd
  partition broadcast -> y^T = Wo^T.T out^T with heads 0/1 row-paired.
"""

import sys

sys.path.insert(0, "/opt/trn_rl_repo")

import numpy as np
from contextlib import ExitStack

import concourse.bass as bass
import concourse.bacc as bacc
import concourse.tile as tile
import concourse.mybir as mybir
from concourse.masks import make_identity
from concourse.bass_utils import run_bass_kernel_spmd

F32 = mybir.dt.float32
AF = mybir.ActivationFunctionType

B = 2
T = 4096
D = 768
H = 12
DK = 64
NCORES = 8
HL = 3  # heads per core
ND = D // 128  # 6 d-tiles
NKT = T // 128  # 32 k-tiles
NQB = T // 512  # 8 q-blocks
NTSB = T // 512  # 8 t-superblocks (4 t-tiles each)

_CACHE = {}
USE_PB = True  # gpsimd partition_broadcast for the reciprocal broadcast


def _emit(tc):
    nc = tc.nc
    x_d = nc.dram_tensor("x", [T, D], F32, kind="ExternalInput").ap()
    wqk_d = nc.dram_tensor("wqkT", [D, 384], F32, kind="ExternalInput").ap()
    wv_d = nc.dram_tensor("wvT", [D, HL * DK], F32, kind="ExternalInput").ap()
    wo_d = nc.dram_tensor("woT", [HL, DK, D], F32, kind="ExternalInput").ap()
    y_d = nc.dram_tensor("yT", [D, T], F32, kind="ExternalOutput").ap()

    ctx = ExitStack()
    const = ctx.enter_context(tc.tile_pool(name="const", bufs=1))
    persist = ctx.enter_context(tc.tile_pool(name="persist", bufs=1))
    xpool = ctx.enter_context(tc.tile_pool(name="xp", bufs=2))
    xtpool = ctx.enter_context(tc.tile_pool(name="xt", bufs=1))
    ptpool = ctx.enter_context(tc.tile_pool(name="pt", bufs=5))
    spool = ctx.enter_context(tc.tile_pool(name="sp", bufs=2))
    otpool = ctx.enter_context(tc.tile_pool(name="ot", bufs=1))
    ypool = ctx.enter_context(tc.tile_pool(name="yp", bufs=2))
    # PSUM: pa = streaming (S tiles, transposes, qkv, V); pb = AV
    # accumulators (one [128,512] bank per head, even/odd halves);
    # pc = sums accumulators + reduce + yT.
    psA = ctx.enter_context(tc.tile_pool(name="psA", bufs=2, space="PSUM"))
    psB = ctx.enter_context(tc.tile_pool(name="psB", bufs=3, space="PSUM"))
    psC = ctx.enter_context(tc.tile_pool(name="psC", bufs=1, space="PSUM"))

    # ---- constants ----
    ident = const.tile([128, 128], F32)
    make_identity(nc, ident)
    # causal band masks for the 4 diagonal-band k-tiles of each q-block:
    # bandmask[bp][k, q] = 0 for q < 128*bp + k, else 1
    bandmask = []
    for bp in range(4):
        m = const.tile([128, 512], F32, name=f"bandmask{bp}")
        nc.gpsimd.memset(m, 1.0)
        nc.gpsimd.affine_select(
            out=m, in_=m, compare_op=mybir.AluOpType.is_ge, fill=0.0,
            base=-128 * bp, pattern=[[1, 512]], channel_multiplier=-1,
        )
        bandmask.append(m)
    ones1 = const.tile([128, 1], F32)
    nc.vector.memset(ones1, 1.0)
    ones64 = const.tile([1, DK], F32)
    nc.vector.memset(ones64, 1.0)
    ones4 = const.tile([128, 1], F32)
    nc.vector.memset(ones4, 0.0)
    for r in (0, 32, 64, 96):
        nc.vector.memset(ones4[r : r + 1, :], 1.0)

    wqk_sb = const.tile([128, ND, 384], F32)
    nc.sync.dma_start(out=wqk_sb, in_=wqk_d.rearrange("(j p) e -> p j e", p=128))
    wv_sb = const.tile([128, ND, HL * DK], F32)
    nc.sync.dma_start(out=wv_sb, in_=wv_d.rearrange("(j p) e -> p j e", p=128))
    wo01_sb = const.tile([128, D], F32)  # head0 rows on 0:64, head1 on 64:128
    nc.sync.dma_start(out=wo01_sb, in_=wo_d[0:2].rearrange("h p d -> (h p) d"))
    wo2_sb = const.tile([DK, D], F32)
    nc.sync.dma_start(out=wo2_sb, in_=wo_d[2])

    # ---- persistent activations ----
    # KA: [K^T_h0 ; K^T_h1], QB: [Q^T_h0 ; Q^T_h1] on partition halves
    KA = persist.tile([128, T], F32, name="KA")
    QB = persist.tile([128, T], F32, name="QB")
    C2 = persist.tile([128, T], F32, name="C2")  # [K^T_h2 ; Q^T_h2]
    D2 = persist.tile([128, T], F32, name="D2")  # [Q^T_h2 ; K^T_h2] (swapped copy)
    Vh = []
    for h in range(HL):
        vt = persist.tile([128, NKT, DK + 1], F32, name=f"V{h}")
        nc.gpsimd.memset(vt[:, :, DK : DK + 1], 1.0)  # ones row -> softmax sums
        Vh.append(vt)
    ot01 = persist.tile([128, 512], F32, name="ot01")  # heads 0/1 out^T per qb
    ot2 = persist.tile([DK, 512], F32, name="ot2")

    qk_dest = [KA, QB, C2]

    # ================= phase A: projections =================
    for tsb in range(NTSB):
        xt_sb = xtpool.tile([128, ND, 512], F32, name="xt_sb")
        for tt in range(4):
            t0 = (tsb * 4 + tt) * 128
            x_sb = xpool.tile([128, D], F32, name="x_sb")
            nc.sync.dma_start(out=x_sb, in_=x_d[t0 : t0 + 128, :])
            ps_t = psA.tile([128, ND * 128], F32, name="ps_t", tag="pa")
            for dj in range(ND):
                nc.tensor.transpose(
                    ps_t[:, dj * 128 : (dj + 1) * 128],
                    x_sb[:, dj * 128 : (dj + 1) * 128],
                    ident,
                )
            nc.vector.tensor_copy(
                xt_sb[:, :, tt * 128 : (tt + 1) * 128],
                ps_t.rearrange("p (j t) -> p j t", j=ND),
            )
        # Q^T / K^T projection: out[e, t] block per e-tile
        for et in range(3):
            ps_q = psA.tile([128, 512], F32, name="ps_q", tag="pa")
            nc.vector.memset(ps_q, 0.0)
            for dj in range(ND):
                e0 = et * 128
                nc.tensor.matmul(
                    ps_q[0:64, :],
                    lhsT=wqk_sb[:, dj, e0 : e0 + 64],
                    rhs=xt_sb[:, dj, :],
                    start=False, stop=(dj == ND - 1), skip_group_check=True,
                )
                nc.tensor.matmul(
                    ps_q[64:128, :],
                    lhsT=wqk_sb[:, dj, e0 + 64 : e0 + 128],
                    rhs=xt_sb[:, dj, :],
                    start=False, stop=(dj == ND - 1), skip_group_check=True,
                )
            nc.vector.tensor_copy(qk_dest[et][:, tsb * 512 : (tsb + 1) * 512], ps_q)
        # D2 = partition-swapped copy of C2 (for self-paired row-tiling of h2)
        blk = slice(tsb * 512, (tsb + 1) * 512)
        nc.sync.dma_start(out=D2[0:64, blk], in_=C2[64:128, blk])
        nc.sync.dma_start(out=D2[64:128, blk], in_=C2[0:64, blk])
        # V natural: stationary x^T tiles, streaming Wv^T
        for tt in range(4):
            ps_v = psA.tile([128, HL * DK], F32, name="ps_v", tag="pa")
            nc.vector.memset(ps_v, 0.0)
            for dj in range(ND):
                tcol = tt * 128
                nc.tensor.matmul(
                    ps_v[0:64, :],
                    lhsT=xt_sb[:, dj, tcol : tcol + 64],
                    rhs=wv_sb[:, dj, :],
                    start=False, stop=(dj == ND - 1), skip_group_check=True,
                )
                nc.tensor.matmul(
                    ps_v[64:128, :],
                    lhsT=xt_sb[:, dj, tcol + 64 : tcol + 128],
                    rhs=wv_sb[:, dj, :],
                    start=False, stop=(dj == ND - 1), skip_group_check=True,
                )
            kt = tsb * 4 + tt
            for h in range(HL):
                nc.vector.tensor_copy(
                    Vh[h][:, kt, 0:DK], ps_v[:, h * DK : (h + 1) * DK]
                )

    # ================= phase B: attention =================
    for qb in range(NQB):
        nk = 4 * (qb + 1)
        qblk = slice(qb * 512, (qb + 1) * 512)
        ot_slices = {}
        # pass 0: heads (0, 1) row-paired; pass 1: head 2 self-paired
        for hpass, heads in enumerate([(0, 1), (2,)]):
            psav = {h: psB.tile([DK + 1, 512], F32, name=f"psav{h}", tag="pb")
                    for h in heads}
            for kp in range(nk // 2):
                kt0, kt1 = 2 * kp, 2 * kp + 1
                ss = {h: psA.tile([128, 1024], F32, name=f"ss{h}", tag="pa")
                      for h in heads}
                for i, kt in enumerate((kt0, kt1)):
                    kblk = slice(kt * 128, (kt + 1) * 128)
                    off = slice(i * 512, (i + 1) * 512)
                    if hpass == 0:
                        nc.tensor.matmul(
                            ss[0][:, off], lhsT=KA[0:64, kblk],
                            rhs=QB[0:64, qblk], start=True, stop=True,
                        )
                        nc.tensor.matmul(
                            ss[1][:, off], lhsT=KA[64:128, kblk],
                            rhs=QB[64:128, qblk], start=True, stop=True,
                        )
                    elif i == 0:
                        nc.tensor.matmul(
                            ss[2][:, off], lhsT=C2[0:64, kblk],
                            rhs=D2[0:64, qblk], start=True, stop=True,
                        )
                    else:
                        nc.tensor.matmul(
                            ss[2][:, off], lhsT=D2[64:128, kblk],
                            rhs=C2[64:128, qblk], start=True, stop=True,
                        )
                for h in heads:
                    pt = ptpool.tile([128, 1024], F32, name="pt")
                    nc.scalar.activation(pt, ss[h], AF.Exp, scale=0.125)
                    for i, kt in enumerate((kt0, kt1)):
                        off = slice(i * 512, (i + 1) * 512)
                        if kt >= 4 * qb:  # diagonal band tile
                            bp = kt - 4 * qb
                            nc.vector.tensor_mul(
                                pt[:, off], pt[:, off], bandmask[bp]
                            )
                        # AV with the ones row appended to V: row 64 of the
                        # accumulator collects the softmax denominators
                        nc.tensor.matmul(
                            psav[h],
                            lhsT=Vh[h][:, kt, :], rhs=pt[:, off],
                            start=(kt == 0), stop=(kt == nk - 1),
                        )
            # normalize: out^T = (even + odd halves) / sums
            for h in heads:
                sums_sb = spool.tile([1, 512], F32, name="sums_sb")
                nc.vector.tensor_copy(sums_sb, psav[h][DK : DK + 1, :])
                chop = spool.tile([128, 4], F32, name="chop")
                nc.sync.dma_start(out=chop, in_=sums_sb)
                recipC = spool.tile([128, 4], F32, name="recipC")
                nc.vector.reciprocal(recipC, chop)
                recipR = spool.tile([1, 512], F32, name="recipR")
                nc.sync.dma_start(out=recipR, in_=recipC)
                recipb = spool.tile([DK, 512], F32, name="recipb")
                if USE_PB:
                    nc.gpsimd.partition_broadcast(recipb, recipR, channels=DK)
                else:
                    ps_b = psC.tile([128, 512], F32, name="ps_b", tag="pc")
                    nc.tensor.matmul(
                        ps_b[0:DK, :], lhsT=ones64, rhs=recipR,
                        start=True, stop=True,
                    )
                    nc.vector.tensor_copy(recipb, ps_b[0:DK, :])
                if h == 0:
                    nc.vector.tensor_mul(ot01[0:DK, :], psav[h][0:DK, :], recipb)
                    ot_slices[0] = ot01[0:DK, :]
                elif h == 1:
                    ot1s = spool.tile([DK, 512], F32, name="ot1s")
                    nc.vector.tensor_mul(ot1s, psav[h][0:DK, :], recipb)
                    nc.sync.dma_start(out=ot01[DK:128, :], in_=ot1s)
                    ot_slices[1] = ot01[DK:128, :]
                else:
                    nc.vector.tensor_mul(ot2, psav[h][0:DK, :], recipb)
                    ot_slices[2] = ot2
        # out-proj: y^T[d, q] — heads 0/1 stacked on partition halves form a
        # single K=128 contraction; then head 2's K=64 accumulates on top.
        # (Mixed ROW positions inside one accumulation group crash the HW,
        # so never pair row-groups within an accumulating chain.)
        for dj in range(ND):
            dblk = slice(dj * 128, (dj + 1) * 128)
            ps_y = psC.tile([128, 512], F32, name="ps_y", tag="pc")
            nc.tensor.matmul(
                ps_y, lhsT=wo01_sb[:, dblk], rhs=ot01,
                start=True, stop=False, skip_group_check=True,
            )
            nc.tensor.matmul(
                ps_y, lhsT=wo2_sb[:, dblk], rhs=ot2,
                start=False, stop=True, skip_group_check=True,
            )
            y_sb = ypool.tile([128, 512], F32, name="y_sb")
            nc.vector.tensor_copy(y_sb, ps_y)
            nc.sync.dma_start(out=y_d[dblk, qblk], in_=y_sb)
    ctx.close()


def build():
    if "nc" in _CACHE:
        return _CACHE["nc"]
    nc = bacc.Bacc(
        "TRN2", target_bir_lowering=False, debug=False, num_devices=NCORES
    )
    with tile.TileContext(nc) as tc:
        _emit(tc)
    nc.compile()
    _CACHE["nc"] = nc
    return nc


def make_in_maps(x, w_qkv, w_out):
    x = np.asarray(x, dtype=np.float32)
    w_qkv = np.asarray(w_qkv, dtype=np.float32)
    w_out = np.asarray(w_out, dtype=np.float32)
    wq = w_qkv[0:D]        # [768, 768], rows = q features
    wk = w_qkv[D : 2 * D]
    wv = w_qkv[2 * D :]
    in_maps = []
    for c in range(NCORES):
        b, g = divmod(c, 4)
        hs = [3 * g + j for j in range(HL)]  # global head ids
        h0, h1, h2 = hs
        cols = []
        for pair in ((wk, h0), (wk, h1), (wq, h0), (wq, h1), (wk, h2), (wq, h2)):
            w, h = pair
            cols.append(w[h * DK : (h + 1) * DK].T)  # [768, 64]
        wqkT = np.ascontiguousarray(np.concatenate(cols, axis=1))  # [768, 384]
        wvT = np.ascontiguousarray(
            np.concatenate([wv[h * DK : (h + 1) * DK].T for h in hs], axis=1)
        )  # [768, 192]
        woT = np.ascontiguousarray(
            np.stack([w_out[:, h * DK : (h + 1) * DK].T for h in hs])
        )  # [3, 64, 768]
        in_maps.append(
            {
                "x": np.ascontiguousarray(x[b]),
                "wqkT": wqkT,
                "wvT": wvT,
                "woT": woT,
            }
        )
    return in_maps


def run(inputs, trace=False):
    """Run on hardware; returns (y [B,T,D] fp32, BassKernelResults)."""
    nc = build()
    in_maps = make_in_maps(inputs["x"], inputs["w_qkv"], inputs["w_out"])
    br = run_bass_kernel_spmd(nc, in_maps, list(range(NCORES)), trace=trace)
    y = np.zeros((B, T, D), dtype=np.float32)
    for c in range(NCORES):
        b = c // 4
        y[b] += np.asarray(br.results[c]["yT"]).T
    return y, br


def kernel(x, w_qkv, w_out):
    y, _ = run({"x": x, "w_qkv": w_qkv, "w_out": w_out})
    return y



# revision 3
# speedup vs baseline: 2.6017x; 2.6017x over previous
"""Multi-head causal self-attention (B=2, T=4096, D=768, H=12) on 8 trn2 cores.

Sharding: core c -> batch b = c//4, heads 3*(c%4) .. 3*(c%4)+2.
qkv_proj column-parallel (each core computes Q/K/V only for its heads),
out_proj row-parallel (each core emits a partial y^T; host sums the 4
partials per batch).

Device dataflow (bf16 operands, fp32 PSUM accumulation):
  x (bf16, pre-cast on host) -> x^T tiles via PE transposes -> Q^T/K^T
  via transposed projection (W^T stationary 128-wide, x^T streaming) ->
  S^T = K Q^T in [k,q] layout, two heads row-paired on opposite PE
  halves -> exp on ScalarE straight out of PSUM into bf16 SBUF (no max
  subtraction; scores ~ N(0,1)) -> causal band masks on DVE (4x-rate
  all-SBUF bf16) -> out^T = V^T P^T with a ones row appended to V
  collecting softmax denominators in accumulator row 64 -> normalize
  via batched reciprocal + gpsimd partition broadcast -> y^T partial =
  Wo^T.T out^T with heads 0/1 row-paired, stored bf16.
"""

import sys

sys.path.insert(0, "/opt/trn_rl_repo")

import numpy as np
import ml_dtypes
from contextlib import ExitStack

import concourse.bass as bass
import concourse.bacc as bacc
import concourse.tile as tile
import concourse.mybir as mybir
from concourse.masks import make_identity
from concourse.bass_utils import run_bass_kernel_spmd

F32 = mybir.dt.float32
BF16 = mybir.dt.bfloat16
AF = mybir.ActivationFunctionType
BF = ml_dtypes.bfloat16

B = 2
T = 4096
D = 768
H = 12
DK = 64
NCORES = 8
HL = 3  # heads per core
ND = D // 128  # 6 d-tiles
NKT = T // 128  # 32 k-tiles
NQB = T // 512  # 8 q-blocks
NTSB = T // 512  # 8 t-superblocks (4 t-tiles each)

_CACHE = {}


def _emit(tc):
    nc = tc.nc
    x_d = nc.dram_tensor("x", [T, D], BF16, kind="ExternalInput").ap()
    wqk_d = nc.dram_tensor("wqkT", [D, 384], BF16, kind="ExternalInput").ap()
    wv_d = nc.dram_tensor("wvT", [D, HL * DK], BF16, kind="ExternalInput").ap()
    wo_d = nc.dram_tensor("woT", [HL, DK, D], BF16, kind="ExternalInput").ap()
    y_d = nc.dram_tensor("yT", [D, T], BF16, kind="ExternalOutput").ap()

    ctx = ExitStack()
    const = ctx.enter_context(tc.tile_pool(name="const", bufs=1))
    persist = ctx.enter_context(tc.tile_pool(name="persist", bufs=1))
    xpool = ctx.enter_context(tc.tile_pool(name="xp", bufs=2))
    xtpool = ctx.enter_context(tc.tile_pool(name="xt", bufs=1))
    ptpool = ctx.enter_context(tc.tile_pool(name="pt", bufs=5))
    spool = ctx.enter_context(tc.tile_pool(name="sp", bufs=2))
    ypool = ctx.enter_context(tc.tile_pool(name="yp", bufs=2))
    # PSUM: pa = streaming (S tiles, transposes, qkv, V); pb = AV
    # accumulators (one bank per head); pc = yT out-proj.
    psA = ctx.enter_context(tc.tile_pool(name="psA", bufs=2, space="PSUM"))
    psB = ctx.enter_context(tc.tile_pool(name="psB", bufs=3, space="PSUM"))
    psC = ctx.enter_context(tc.tile_pool(name="psC", bufs=1, space="PSUM"))

    # ---- constants ----
    ident = const.tile([128, 128], BF16)
    make_identity(nc, ident)
    # causal band masks for the 4 diagonal-band k-tiles of each q-block:
    # bandmask[bp][k, q] = 0 for q < 128*bp + k, else 1
    bandmask = []
    for bp in range(4):
        m = const.tile([128, 512], BF16, name=f"bandmask{bp}")
        nc.gpsimd.memset(m, 1.0)
        nc.gpsimd.affine_select(
            out=m, in_=m, compare_op=mybir.AluOpType.is_ge, fill=0.0,
            base=-128 * bp, pattern=[[1, 512]], channel_multiplier=-1,
        )
        bandmask.append(m)
    ones64 = const.tile([1, DK], F32)
    nc.vector.memset(ones64, 1.0)

    wqk_sb = const.tile([128, ND, 384], BF16)
    nc.sync.dma_start(out=wqk_sb, in_=wqk_d.rearrange("(j p) e -> p j e", p=128))
    wv_sb = const.tile([128, ND, HL * DK], BF16)
    nc.sync.dma_start(out=wv_sb, in_=wv_d.rearrange("(j p) e -> p j e", p=128))
    wo01_sb = const.tile([128, D], BF16)  # head0 rows on 0:64, head1 on 64:128
    nc.sync.dma_start(out=wo01_sb, in_=wo_d[0:2].rearrange("h p d -> (h p) d"))
    wo2_sb = const.tile([DK, D], BF16)
    nc.sync.dma_start(out=wo2_sb, in_=wo_d[2])

    # ---- persistent activations ----
    # KA: [K^T_h0 ; K^T_h1], QB: [Q^T_h0 ; Q^T_h1] on partition halves
    KA = persist.tile([128, T], BF16, name="KA")
    QB = persist.tile([128, T], BF16, name="QB")
    C2 = persist.tile([128, T], BF16, name="C2")  # [K^T_h2 ; Q^T_h2]
    D2 = persist.tile([128, T], BF16, name="D2")  # [Q^T_h2 ; K^T_h2] (swapped)
    Vh = []
    for h in range(HL):
        vt = persist.tile([128, NKT, DK + 1], BF16, name=f"V{h}")
        nc.gpsimd.memset(vt[:, :, DK : DK + 1], 1.0)  # ones row -> softmax sums
        Vh.append(vt)
    ot01 = persist.tile([128, 512], BF16, name="ot01")  # heads 0/1 out^T per qb
    ot2 = persist.tile([DK, 512], BF16, name="ot2")

    qk_dest = [KA, QB, C2]

    # ================= phase A: projections =================
    for tsb in range(NTSB):
        xt_sb = xtpool.tile([128, ND, 512], BF16, name="xt_sb")
        for tt in range(4):
            t0 = (tsb * 4 + tt) * 128
            x_sb = xpool.tile([128, D], BF16, name="x_sb")
            nc.sync.dma_start(out=x_sb, in_=x_d[t0 : t0 + 128, :])
            ps_t = psA.tile([128, ND * 128], BF16, name="ps_t", tag="pa")
            for dj in range(ND):
                nc.tensor.transpose(
                    ps_t[:, dj * 128 : (dj + 1) * 128],
                    x_sb[:, dj * 128 : (dj + 1) * 128],
                    ident,
                )
            nc.vector.tensor_copy(
                xt_sb[:, :, tt * 128 : (tt + 1) * 128],
                ps_t.rearrange("p (j t) -> p j t", j=ND),
            )
        # Q^T / K^T projection: out[e, t] block per e-tile (full 128-wide
        # stationary: e-tile 0 = [K_h0|K_h1], 1 = [Q_h0|Q_h1], 2 = [K_h2|Q_h2])
        for et in range(3):
            ps_q = psA.tile([128, 512], F32, name="ps_q", tag="pa")
            e0 = et * 128
            for dj in range(ND):
                nc.tensor.matmul(
                    ps_q,
                    lhsT=wqk_sb[:, dj, e0 : e0 + 128],
                    rhs=xt_sb[:, dj, :],
                    start=(dj == 0), stop=(dj == ND - 1),
                )
            nc.vector.tensor_copy(qk_dest[et][:, tsb * 512 : (tsb + 1) * 512], ps_q)
        # D2 = partition-swapped copy of C2 (for self-paired row-tiling of h2)
        blk = slice(tsb * 512, (tsb + 1) * 512)
        nc.sync.dma_start(out=D2[0:64, blk], in_=C2[64:128, blk])
        nc.sync.dma_start(out=D2[64:128, blk], in_=C2[0:64, blk])
        # V natural: stationary x^T tiles (full 128-wide), streaming Wv^T
        for tt in range(4):
            ps_v = psA.tile([128, HL * DK], F32, name="ps_v", tag="pa")
            tcol = tt * 128
            for dj in range(ND):
                nc.tensor.matmul(
                    ps_v,
                    lhsT=xt_sb[:, dj, tcol : tcol + 128],
                    rhs=wv_sb[:, dj, :],
                    start=(dj == 0), stop=(dj == ND - 1),
                )
            kt = tsb * 4 + tt
            for h in range(HL):
                nc.vector.tensor_copy(
                    Vh[h][:, kt, 0:DK], ps_v[:, h * DK : (h + 1) * DK]
                )

    # ================= phase B: attention =================
    for qb in range(NQB):
        nk = 4 * (qb + 1)
        qblk = slice(qb * 512, (qb + 1) * 512)
        # pass 0: heads (0, 1) row-paired; pass 1: head 2 self-paired
        for hpass, heads in enumerate([(0, 1), (2,)]):
            psav = {h: psB.tile([DK + 1, 512], F32, name=f"psav{h}", tag="pb")
                    for h in heads}
            for kp in range(nk // 2):
                kt0, kt1 = 2 * kp, 2 * kp + 1
                ss = {h: psA.tile([128, 1024], F32, name=f"ss{h}", tag="pa")
                      for h in heads}
                for i, kt in enumerate((kt0, kt1)):
                    kblk = slice(kt * 128, (kt + 1) * 128)
                    off = slice(i * 512, (i + 1) * 512)
                    if hpass == 0:
                        nc.tensor.matmul(
                            ss[0][:, off], lhsT=KA[0:64, kblk],
                            rhs=QB[0:64, qblk], start=True, stop=True,
                        )
                        nc.tensor.matmul(
                            ss[1][:, off], lhsT=KA[64:128, kblk],
                            rhs=QB[64:128, qblk], start=True, stop=True,
                        )
                    elif i == 0:
                        nc.tensor.matmul(
                            ss[2][:, off], lhsT=C2[0:64, kblk],
                            rhs=D2[0:64, qblk], start=True, stop=True,
                        )
                    else:
                        nc.tensor.matmul(
                            ss[2][:, off], lhsT=D2[64:128, kblk],
                            rhs=C2[64:128, qblk], start=True, stop=True,
                        )
                for h in heads:
                    pt = ptpool.tile([128, 1024], BF16, name="pt")
                    nc.scalar.activation(pt, ss[h], AF.Exp, scale=0.125)
                    for i, kt in enumerate((kt0, kt1)):
                        off = slice(i * 512, (i + 1) * 512)
                        if kt >= 4 * qb:  # diagonal band tile
                            bp = kt - 4 * qb
                            nc.vector.tensor_mul(
                                pt[:, off], pt[:, off], bandmask[bp]
                            )
                        # AV with the ones row appended to V: row 64 of the
                        # accumulator collects the softmax denominators
                        nc.tensor.matmul(
                            psav[h],
                            lhsT=Vh[h][:, kt, :], rhs=pt[:, off],
                            start=(kt == 0), stop=(kt == nk - 1),
                        )
            # normalize: out^T = psav / sums
            for h in heads:
                sums_sb = spool.tile([1, 512], F32, name="sums_sb")
                nc.vector.tensor_copy(sums_sb, psav[h][DK : DK + 1, :])
                chop = spool.tile([128, 4], F32, name="chop")
                nc.sync.dma_start(out=chop, in_=sums_sb)
                recipC = spool.tile([128, 4], F32, name="recipC")
                nc.vector.reciprocal(recipC, chop)
                recipR = spool.tile([1, 512], F32, name="recipR")
                nc.sync.dma_start(out=recipR, in_=recipC)
                recipb = spool.tile([DK, 512], F32, name="recipb")
                nc.gpsimd.partition_broadcast(recipb, recipR, channels=DK)
                if h == 0:
                    nc.vector.tensor_mul(ot01[0:DK, :], psav[h][0:DK, :], recipb)
                elif h == 1:
                    ot1s = spool.tile([DK, 512], BF16, name="ot1s")
                    nc.vector.tensor_mul(ot1s, psav[h][0:DK, :], recipb)
                    nc.sync.dma_start(out=ot01[DK:128, :], in_=ot1s)
                else:
                    nc.vector.tensor_mul(ot2, psav[h][0:DK, :], recipb)
        # out-proj: y^T[d, q] — heads 0/1 stacked on partition halves form a
        # single K=128 contraction; then head 2's K=64 accumulates on top.
        # (Mixed ROW positions inside one accumulation group crash the HW,
        # so never pair row-groups within an accumulating chain.)
        for dj in range(ND):
            dblk = slice(dj * 128, (dj + 1) * 128)
            ps_y = psC.tile([128, 512], F32, name="ps_y", tag="pc")
            nc.tensor.matmul(
                ps_y, lhsT=wo01_sb[:, dblk], rhs=ot01,
                start=True, stop=False, skip_group_check=True,
            )
            nc.tensor.matmul(
                ps_y, lhsT=wo2_sb[:, dblk], rhs=ot2,
                start=False, stop=True, skip_group_check=True,
            )
            y_sb = ypool.tile([128, 512], BF16, name="y_sb")
            nc.vector.tensor_copy(y_sb, ps_y)
            nc.sync.dma_start(out=y_d[dblk, qblk], in_=y_sb)
    ctx.close()


def build():
    if "nc" in _CACHE:
        return _CACHE["nc"]
    nc = bacc.Bacc(
        "TRN2", target_bir_lowering=False, debug=False, num_devices=NCORES
    )
    with tile.TileContext(nc) as tc:
        _emit(tc)
    nc.compile()
    _CACHE["nc"] = nc
    return nc


def make_in_maps(x, w_qkv, w_out):
    x = np.asarray(x, dtype=np.float32)
    w_qkv = np.asarray(w_qkv, dtype=np.float32)
    w_out = np.asarray(w_out, dtype=np.float32)
    wq = w_qkv[0:D]        # [768, 768], rows = q features
    wk = w_qkv[D : 2 * D]
    wv = w_qkv[2 * D :]
    in_maps = []
    for c in range(NCORES):
        b, g = divmod(c, 4)
        hs = [3 * g + j for j in range(HL)]  # global head ids
        h0, h1, h2 = hs
        cols = []
        for pair in ((wk, h0), (wk, h1), (wq, h0), (wq, h1), (wk, h2), (wq, h2)):
            w, h = pair
            cols.append(w[h * DK : (h + 1) * DK].T)  # [768, 64]
        wqkT = np.ascontiguousarray(np.concatenate(cols, axis=1))  # [768, 384]
        wvT = np.ascontiguousarray(
            np.concatenate([wv[h * DK : (h + 1) * DK].T for h in hs], axis=1)
        )  # [768, 192]
        woT = np.ascontiguousarray(
            np.stack([w_out[:, h * DK : (h + 1) * DK].T for h in hs])
        )  # [3, 64, 768]
        in_maps.append(
            {
                "x": np.ascontiguousarray(x[b]).astype(BF),
                "wqkT": wqkT.astype(BF),
                "wvT": wvT.astype(BF),
                "woT": woT.astype(BF),
            }
        )
    return in_maps


def run(inputs, trace=False):
    """Run on hardware; returns (y [B,T,D] fp32, BassKernelResults)."""
    nc = build()
    in_maps = make_in_maps(inputs["x"], inputs["w_qkv"], inputs["w_out"])
    br = run_bass_kernel_spmd(nc, in_maps, list(range(NCORES)), trace=trace)
    y = np.zeros((B, T, D), dtype=np.float32)
    for c in range(NCORES):
        b = c // 4
        y[b] += np.asarray(br.results[c]["yT"]).astype(np.float32).T
    return y, br


def kernel(x, w_qkv, w_out):
    y, _ = run({"x": x, "w_qkv": w_qkv, "w_out": w_out})
    return y


# revision 12
# speedup vs baseline: 2.7469x; 1.0558x over previous
"""Multi-head causal self-attention (B=2, T=4096, D=768, H=12) on 8 trn2 cores.

Sharding: core c -> batch b = c//4, heads 3*(c%4) .. 3*(c%4)+2.
qkv_proj column-parallel (each core computes Q/K/V only for its heads),
out_proj row-parallel (each core emits a partial y^T; host sums the 4
partials per batch).

Device dataflow (bf16 operands, fp32 PSUM accumulation):
  x^T pre-transposed and cast to bf16 on the host -> Q^T/K^T via
  transposed projection (W^T stationary 128-wide, x^T streaming) ->
  S^T = K Q^T in [k,q] layout, two heads row-paired on opposite PE
  halves -> exp on ScalarE straight out of PSUM into bf16 SBUF (no max
  subtraction; scores ~ N(0,1)) -> causal band masks on DVE (4x-rate
  all-SBUF bf16) -> out^T = V^T P^T with a ones column appended to V
  collecting softmax denominators (head1's accumulator sits on
  partitions 63:128 with the ones column first, so its normalized
  output lands directly on ot01[64:128] without a bounce DMA) ->
  normalize via approx reciprocal + gpsimd partition broadcast ->
  y^T partial = Wo^T.T out^T with heads 0/1 row-paired, stored bf16.

Emission interleaves projection superblocks with attention q-blocks
(proj tsb=s, attn qb=s) so ScalarE exp overlaps projection matmuls,
and AV matmuls lag one k-pair behind scores so the PE queue never
head-of-line blocks on the exp."""

import sys

sys.path.insert(0, "/opt/trn_rl_repo")

import numpy as np
import ml_dtypes
from contextlib import ExitStack

import concourse.bass as bass
import concourse.bacc as bacc
import concourse.tile as tile
import concourse.mybir as mybir
from concourse.bass_utils import run_bass_kernel_spmd

F32 = mybir.dt.float32
BF16 = mybir.dt.bfloat16
AF = mybir.ActivationFunctionType
BF = ml_dtypes.bfloat16

B = 2
T = 4096
D = 768
H = 12
DK = 64
NCORES = 8
HL = 3  # heads per core
ND = D // 128  # 6 d-tiles
NKT = T // 128  # 32 k-tiles
NQB = T // 512  # 8 q-blocks
NTSB = T // 512  # 8 t-superblocks (4 t-tiles each)

_CACHE = {}


def _emit(tc):
    nc = tc.nc
    xt_d = nc.dram_tensor("xT", [D, T], BF16, kind="ExternalInput").ap()
    wqk_d = nc.dram_tensor("wqkT", [D, 384], BF16, kind="ExternalInput").ap()
    wv_d = nc.dram_tensor("wvT", [D, HL * DK], BF16, kind="ExternalInput").ap()
    wo_d = nc.dram_tensor("woT", [HL, DK, D], BF16, kind="ExternalInput").ap()
    y_d = nc.dram_tensor("yT", [D, T], BF16, kind="ExternalOutput").ap()
    xt_v = xt_d.rearrange("(j p) t -> p j t", p=128)

    ctx = ExitStack()
    const = ctx.enter_context(tc.tile_pool(name="const", bufs=1))
    persist = ctx.enter_context(tc.tile_pool(name="persist", bufs=1))
    xtpool = ctx.enter_context(tc.tile_pool(name="xt", bufs=2))
    ptpool = ctx.enter_context(tc.tile_pool(name="pt", bufs=5))
    spool = ctx.enter_context(tc.tile_pool(name="sp", bufs=2))
    ypool = ctx.enter_context(tc.tile_pool(name="yp", bufs=2))
    # PSUM: pa = streaming (S tiles, qkv, V); pb = AV accumulators (one
    # bank per head); pc = yT out-proj.
    psA = ctx.enter_context(tc.tile_pool(name="psA", bufs=2, space="PSUM"))
    psB = ctx.enter_context(tc.tile_pool(name="psB", bufs=3, space="PSUM"))
    psC = ctx.enter_context(tc.tile_pool(name="psC", bufs=1, space="PSUM"))

    # ---- constants ----
    # causal band masks for the 4 diagonal-band k-tiles of each q-block:
    # bandmask[bp][k, q] = 0 for q < 128*bp + k, else 1
    bandmask = []
    for bp in range(4):
        m = const.tile([128, 512], BF16, name=f"bandmask{bp}")
        nc.gpsimd.memset(m, 1.0)
        nc.gpsimd.affine_select(
            out=m, in_=m, compare_op=mybir.AluOpType.is_ge, fill=0.0,
            base=-128 * bp, pattern=[[1, 512]], channel_multiplier=-1,
        )
        bandmask.append(m)

    wqk_sb = const.tile([128, ND, 384], BF16)
    nc.sync.dma_start(out=wqk_sb, in_=wqk_d.rearrange("(j p) e -> p j e", p=128))
    wv_sb = const.tile([128, ND, HL * DK], BF16)
    nc.sync.dma_start(out=wv_sb, in_=wv_d.rearrange("(j p) e -> p j e", p=128))
    wo01_sb = const.tile([128, D], BF16)  # head0 rows on 0:64, head1 on 64:128
    nc.sync.dma_start(out=wo01_sb, in_=wo_d[0:2].rearrange("h p d -> (h p) d"))
    wo2_sb = const.tile([DK, D], BF16)
    nc.sync.dma_start(out=wo2_sb, in_=wo_d[2])

    # ---- persistent activations ----
    # KA: [K^T_h0 ; K^T_h1], QB: [Q^T_h0 ; Q^T_h1] on partition halves
    KA = persist.tile([128, T], BF16, name="KA")
    QB = persist.tile([128, T], BF16, name="QB")
    C2 = persist.tile([128, T], BF16, name="C2")  # [K^T_h2 ; Q^T_h2]
    D2 = persist.tile([128, T], BF16, name="D2")  # [Q^T_h2 ; K^T_h2] (swapped)
    # V: [128 k-part, kt, head, 65]; per head a 65-col block of [v, ones]
    # (the ones column collects softmax denominators in accumulator row 64)
    Vh = []
    for h in range(HL):
        vt = persist.tile([128, NKT, DK + 1], BF16, name=f"V{h}")
        nc.gpsimd.memset(vt[:, :, DK : DK + 1], 1.0)  # ones col -> softmax sums
        Vh.append(vt)
    ot01 = persist.tile([128, 512], BF16, name="ot01")  # heads 0/1 out^T per qb
    ot2 = persist.tile([DK, 512], BF16, name="ot2")

    qk_dest = [KA, QB, C2]

    def emit_proj(tsb):
        blk = slice(tsb * 512, (tsb + 1) * 512)
        xt_sb = xtpool.tile([128, ND, 512], BF16, name="xt_sb")
        nc.sync.dma_start(out=xt_sb, in_=xt_v[:, :, blk])
        # Q^T / K^T projection: out[e, t] block per e-tile (full 128-wide
        # stationary: e-tile 0 = [K_h0|K_h1], 1 = [Q_h0|Q_h1], 2 = [K_h2|Q_h2])
        for et in range(3):
            ps_q = psA.tile([128, 512], F32, name="ps_q", tag="pa")
            e0 = et * 128
            for dj in range(ND):
                nc.tensor.matmul(
                    ps_q,
                    lhsT=wqk_sb[:, dj, e0 : e0 + 128],
                    rhs=xt_sb[:, dj, :],
                    start=(dj == 0), stop=(dj == ND - 1),
                )
            nc.vector.tensor_copy(qk_dest[et][:, blk], ps_q)
        # D2 = partition-swapped copy of C2 (for self-paired row-tiling of h2)
        nc.sync.dma_start(out=D2[0:64, blk], in_=C2[64:128, blk])
        nc.sync.dma_start(out=D2[64:128, blk], in_=C2[0:64, blk])
        # V natural: stationary x^T tiles (full 128-wide), streaming Wv^T
        for tt in range(4):
            ps_v = psA.tile([128, HL * DK], F32, name="ps_v", tag="pa")
            tcol = tt * 128
            for dj in range(ND):
                nc.tensor.matmul(
                    ps_v,
                    lhsT=xt_sb[:, dj, tcol : tcol + 128],
                    rhs=wv_sb[:, dj, :],
                    start=(dj == 0), stop=(dj == ND - 1),
                )
            kt = tsb * 4 + tt
            for h in range(HL):
                nc.vector.tensor_copy(
                    Vh[h][:, kt, 0:DK], ps_v[:, h * DK : (h + 1) * DK]
                )

    def emit_attn(qb):
        nk = 4 * (qb + 1)
        qblk = slice(qb * 512, (qb + 1) * 512)
        # pass 0: heads (0, 1) row-paired; pass 1: head 2 self-paired
        for hpass, heads in enumerate([(0, 1), (2,)]):
            psav = {h: psB.tile([DK + 1, 512], F32, name=f"psav{h}", tag="pb")
                    for h in heads}

            def emit_av(kts, pts):
                for h in heads:
                    for i, kt in enumerate(kts):
                        off = slice(i * 512, (i + 1) * 512)
                        nc.tensor.matmul(
                            psav[h],
                            lhsT=Vh[h][:, kt, :], rhs=pts[h][:, off],
                            start=(kt == 0), stop=(kt == nk - 1),
                        )

            pend = None
            for kp in range(nk // 2):
                kt0, kt1 = 2 * kp, 2 * kp + 1
                ss = {h: psA.tile([128, 1024], F32, name=f"ss{h}", tag="pa")
                      for h in heads}
                for i, kt in enumerate((kt0, kt1)):
                    kblk = slice(kt * 128, (kt + 1) * 128)
                    off = slice(i * 512, (i + 1) * 512)
                    if hpass == 0:
                        nc.tensor.matmul(
                            ss[0][:, off], lhsT=KA[0:64, kblk],
                            rhs=QB[0:64, qblk], start=True, stop=True,
                        )
                        nc.tensor.matmul(
                            ss[1][:, off], lhsT=KA[64:128, kblk],
                            rhs=QB[64:128, qblk], start=True, stop=True,
                        )
                    elif i == 0:
                        nc.tensor.matmul(
                            ss[2][:, off], lhsT=C2[0:64, kblk],
                            rhs=D2[0:64, qblk], start=True, stop=True,
                        )
                    else:
                        nc.tensor.matmul(
                            ss[2][:, off], lhsT=D2[64:128, kblk],
                            rhs=C2[64:128, qblk], start=True, stop=True,
                        )
                pts = {}
                for h in heads:
                    pt = ptpool.tile([128, 1024], BF16, name="pt")
                    nc.scalar.activation(pt, ss[h], AF.Exp, scale=0.125)
                    for i, kt in enumerate((kt0, kt1)):
                        if kt >= 4 * qb:  # diagonal band tile
                            off = slice(i * 512, (i + 1) * 512)
                            nc.vector.tensor_mul(
                                pt[:, off], pt[:, off], bandmask[kt - 4 * qb]
                            )
                    pts[h] = pt
                if pend is not None:
                    emit_av(*pend)
                pend = ((kt0, kt1), pts)
            emit_av(*pend)
            # normalize: out^T = psav / sums (sums sit in ones row 64).
            # (reciprocal_approx_fast is a custom DVE op and must read SBUF,
            # not PSUM — feeding it psav directly returns garbage.)
            for h in heads:
                pa = psav[h]
                sums_sb = spool.tile([1, 512], F32, name="sums_sb")
                nc.vector.tensor_copy(sums_sb, pa[DK : DK + 1, :])
                rt = spool.tile([1, 512], F32, name="rt")
                nc.vector.reciprocal_approx_fast(rt, sums_sb)
                rb = spool.tile([DK, 512], F32, name="rb")
                nc.gpsimd.partition_broadcast(rb, rt, channels=DK)
                if h == 0:
                    nc.vector.tensor_mul(ot01[0:DK, :], pa[0:DK, :], rb)
                elif h == 1:
                    ot1s = spool.tile([DK, 512], BF16, name="ot1s")
                    nc.vector.tensor_mul(ot1s, pa[0:DK, :], rb)
                    nc.sync.dma_start(out=ot01[DK:128, :], in_=ot1s)
                else:
                    nc.vector.tensor_mul(ot2, pa[0:DK, :], rb)
        # out-proj: y^T[d, q] — heads 0/1 stacked on partition halves form a
        # single K=128 contraction; then head 2's K=64 accumulates on top.
        # (Mixed ROW positions inside one accumulation group crash the HW,
        # so never pair row-groups within an accumulating chain.)
        for dj in range(ND):
            dblk = slice(dj * 128, (dj + 1) * 128)
            ps_y = psC.tile([128, 512], F32, name="ps_y", tag="pc")
            nc.tensor.matmul(
                ps_y, lhsT=wo01_sb[:, dblk], rhs=ot01,
                start=True, stop=False, skip_group_check=True,
            )
            nc.tensor.matmul(
                ps_y, lhsT=wo2_sb[:, dblk], rhs=ot2,
                start=False, stop=True, skip_group_check=True,
            )
            y_sb = ypool.tile([128, 512], BF16, name="y_sb")
            nc.vector.tensor_copy(y_sb, ps_y)
            nc.sync.dma_start(out=y_d[dblk, qblk], in_=y_sb)

    for step in range(NTSB):
        emit_proj(step)
        emit_attn(step)
    ctx.close()


def build():
    if "nc" in _CACHE:
        return _CACHE["nc"]
    nc = bacc.Bacc(
        "TRN2", target_bir_lowering=False, debug=False, num_devices=NCORES
    )
    with tile.TileContext(nc) as tc:
        _emit(tc)
    nc.compile()
    _CACHE["nc"] = nc
    return nc


def make_in_maps(x, w_qkv, w_out):
    x = np.asarray(x, dtype=np.float32)
    w_qkv = np.asarray(w_qkv, dtype=np.float32)
    w_out = np.asarray(w_out, dtype=np.float32)
    wq = w_qkv[0:D]        # [768, 768], rows = q features
    wk = w_qkv[D : 2 * D]
    wv = w_qkv[2 * D :]
    xT = [np.ascontiguousarray(x[b].T).astype(BF) for b in range(B)]
    in_maps = []
    for c in range(NCORES):
        b, g = divmod(c, 4)
        hs = [3 * g + j for j in range(HL)]  # global head ids
        h0, h1, h2 = hs
        cols = []
        for pair in ((wk, h0), (wk, h1), (wq, h0), (wq, h1), (wk, h2), (wq, h2)):
            w, h = pair
            cols.append(w[h * DK : (h + 1) * DK].T)  # [768, 64]
        wqkT = np.ascontiguousarray(np.concatenate(cols, axis=1))  # [768, 384]
        wvT = np.ascontiguousarray(
            np.concatenate([wv[h * DK : (h + 1) * DK].T for h in hs], axis=1)
        )  # [768, 192]
        woT = np.ascontiguousarray(
            np.stack([w_out[:, h * DK : (h + 1) * DK].T for h in hs])
        )  # [3, 64, 768]
        in_maps.append(
            {
                "xT": xT[b],
                "wqkT": wqkT.astype(BF),
                "wvT": wvT.astype(BF),
                "woT": woT.astype(BF),
            }
        )
    return in_maps


def run(inputs, trace=False):
    """Run on hardware; returns (y [B,T,D] fp32, BassKernelResults)."""
    nc = build()
    in_maps = make_in_maps(inputs["x"], inputs["w_qkv"], inputs["w_out"])
    br = run_bass_kernel_spmd(nc, in_maps, list(range(NCORES)), trace=trace)
    y = np.zeros((B, T, D), dtype=np.float32)
    for c in range(NCORES):
        b = c // 4
        y[b] += np.asarray(br.results[c]["yT"]).astype(np.float32).T
    return y, br


def kernel(x, w_qkv, w_out):
    y, _ = run({"x": x, "w_qkv": w_qkv, "w_out": w_out})
    return y


# revision 13
# speedup vs baseline: 2.9493x; 1.0737x over previous
"""Multi-head causal self-attention (B=2, T=4096, D=768, H=12) on 8 trn2 cores.

Sharding: core c -> batch b = c//4, heads 3*(c%4) .. 3*(c%4)+2.
qkv_proj column-parallel (each core computes Q/K/V only for its heads),
out_proj row-parallel (each core emits a partial y^T; host sums the 4
partials per batch).

Device dataflow (bf16 operands, fp32 PSUM accumulation):
  x^T pre-transposed and cast to bf16 on the host -> Q^T/K^T via
  transposed projection (W^T stationary 128-wide, x^T streaming) ->
  S^T = K Q^T in [k,q] layout, two heads row-paired on opposite PE
  halves -> exp straight out of PSUM into bf16 SBUF; heads 0/1 use
  ScalarE Exp, head 2 alternates ScalarE with a DVE integer-bitcast
  2^u approximation (one tensor_scalar into int16 whose bits are the
  bf16 exp; +-3% sawtooth cancels in the softmax normalization) ->
  causal band masks on DVE (4x-rate all-SBUF bf16) -> out^T = V^T P^T
  with a ones column appended to V collecting softmax denominators ->
  normalize via approx reciprocal + gpsimd partition broadcast ->
  y^T partial = Wo^T.T out^T with heads 0/1 row-paired, stored bf16.

Emission order per step: attention hpass0 (h0/h1) -> normalize h0/h1
-> hpass1 (h2) -> normalize h2 -> projection for the NEXT superblock
-> out-proj of this q-block. The PE queue therefore always has
projection work while the normalize chain runs, and AV matmuls lag one
k-pair behind scores so the PE never head-of-line blocks on exp."""

import sys

sys.path.insert(0, "/opt/trn_rl_repo")

import numpy as np
import ml_dtypes
from contextlib import ExitStack

import concourse.bass as bass
import concourse.bacc as bacc
import concourse.tile as tile
import concourse.mybir as mybir
from concourse.bass_utils import run_bass_kernel_spmd

F32 = mybir.dt.float32
BF16 = mybir.dt.bfloat16
I16 = mybir.dt.int16
AF = mybir.ActivationFunctionType
ALU = mybir.AluOpType
BF = ml_dtypes.bfloat16

B = 2
T = 4096
D = 768
H = 12
DK = 64
NCORES = 8
HL = 3  # heads per core
ND = D // 128  # 6 d-tiles
NKT = T // 128  # 32 k-tiles
NQB = T // 512  # 8 q-blocks
NTSB = T // 512  # 8 t-superblocks (4 t-tiles each)

# DVE bitcast-exp constants: bf16bits(exp(s)) ~ round(s*0.125*log2e*128
# + (16256 - 5.5)); the -5.5 centers the linear-mantissa sawtooth.
EXP_C1 = 0.125 * 1.4426950408889634 * 128.0
EXP_C2 = 16256.0 - 5.5

_CACHE = {}


def _emit(tc):
    nc = tc.nc
    xt_d = nc.dram_tensor("xT", [D, T], BF16, kind="ExternalInput").ap()
    wqk_d = nc.dram_tensor("wqkT", [D, 384], BF16, kind="ExternalInput").ap()
    wv_d = nc.dram_tensor("wvT", [D, HL * DK], BF16, kind="ExternalInput").ap()
    wo_d = nc.dram_tensor("woT", [HL, DK, D], BF16, kind="ExternalInput").ap()
    y_d = nc.dram_tensor("yT", [D, T], BF16, kind="ExternalOutput").ap()
    xt_v = xt_d.rearrange("(j p) t -> p j t", p=128)
    y_v = y_d.rearrange("(j p) t -> p j t", p=128)

    ctx = ExitStack()
    const = ctx.enter_context(tc.tile_pool(name="const", bufs=1))
    persist = ctx.enter_context(tc.tile_pool(name="persist", bufs=1))
    xtpool = ctx.enter_context(tc.tile_pool(name="xt", bufs=2))
    ptpool = ctx.enter_context(tc.tile_pool(name="pt", bufs=5))
    spool = ctx.enter_context(tc.tile_pool(name="sp", bufs=2))
    ypool = ctx.enter_context(tc.tile_pool(name="yp", bufs=2))
    # PSUM (8 banks): pa = streaming (S tiles, qkv, V) 2 bufs x 2 banks;
    # pb = AV accumulators, 2 bufs (h2 reuses h0's slot after normalize
    # h0); pc = yT out-proj, 2 bufs so dj pipelines across the copy.
    psA = ctx.enter_context(tc.tile_pool(name="psA", bufs=2, space="PSUM"))
    psB = ctx.enter_context(tc.tile_pool(name="psB", bufs=2, space="PSUM"))
    psC = ctx.enter_context(tc.tile_pool(name="psC", bufs=2, space="PSUM"))

    # ---- constants ----
    # causal band masks for the 4 diagonal-band k-tiles of each q-block:
    # bandmask[bp][k, q] = 0 for q < 128*bp + k, else 1
    bandmask = []
    for bp in range(4):
        m = const.tile([128, 512], BF16, name=f"bandmask{bp}")
        nc.gpsimd.memset(m, 1.0)
        nc.gpsimd.affine_select(
            out=m, in_=m, compare_op=mybir.AluOpType.is_ge, fill=0.0,
            base=-128 * bp, pattern=[[1, 512]], channel_multiplier=-1,
        )
        bandmask.append(m)

    wqk_sb = const.tile([128, ND, 384], BF16)
    nc.sync.dma_start(out=wqk_sb, in_=wqk_d.rearrange("(j p) e -> p j e", p=128))
    wv_sb = const.tile([128, ND, HL * DK], BF16)
    nc.sync.dma_start(out=wv_sb, in_=wv_d.rearrange("(j p) e -> p j e", p=128))
    wo01_sb = const.tile([128, D], BF16)  # head0 rows on 0:64, head1 on 64:128
    nc.sync.dma_start(out=wo01_sb, in_=wo_d[0:2].rearrange("h p d -> (h p) d"))
    wo2_sb = const.tile([DK, D], BF16)
    nc.sync.dma_start(out=wo2_sb, in_=wo_d[2])

    # ---- persistent activations ----
    # KA: [K^T_h0 ; K^T_h1], QB: [Q^T_h0 ; Q^T_h1] on partition halves
    KA = persist.tile([128, T], BF16, name="KA")
    QB = persist.tile([128, T], BF16, name="QB")
    C2 = persist.tile([128, T], BF16, name="C2")  # [K^T_h2 ; Q^T_h2]
    D2 = persist.tile([128, T], BF16, name="D2")  # [Q^T_h2 ; K^T_h2] (swapped)
    # V natural layout per head + a ones column collecting softmax sums
    Vh = []
    for h in range(HL):
        vt = persist.tile([128, NKT, DK + 1], BF16, name=f"V{h}")
        nc.gpsimd.memset(vt[:, :, DK : DK + 1], 1.0)
        Vh.append(vt)
    ot01 = persist.tile([128, 512], BF16, name="ot01")  # heads 0/1 out^T per qb
    ot2 = persist.tile([DK, 512], BF16, name="ot2")

    qk_dest = [KA, QB, C2]

    def emit_proj(tsb):
        blk = slice(tsb * 512, (tsb + 1) * 512)
        xt_sb = xtpool.tile([128, ND, 512], BF16, name="xt_sb")
        nc.sync.dma_start(out=xt_sb, in_=xt_v[:, :, blk])
        # Q^T / K^T projection: out[e, t] block per e-tile (full 128-wide
        # stationary: e-tile 0 = [K_h0|K_h1], 1 = [Q_h0|Q_h1], 2 = [K_h2|Q_h2])
        for et in range(3):
            ps_q = psA.tile([128, 512], F32, name="ps_q", tag="pa")
            e0 = et * 128
            for dj in range(ND):
                nc.tensor.matmul(
                    ps_q,
                    lhsT=wqk_sb[:, dj, e0 : e0 + 128],
                    rhs=xt_sb[:, dj, :],
                    start=(dj == 0), stop=(dj == ND - 1),
                )
            nc.vector.tensor_copy(qk_dest[et][:, blk], ps_q)
        # D2 = partition-swapped copy of C2 (for self-paired row-tiling of h2)
        nc.sync.dma_start(out=D2[0:64, blk], in_=C2[64:128, blk])
        nc.sync.dma_start(out=D2[64:128, blk], in_=C2[0:64, blk])
        # V natural: stationary x^T tiles (full 128-wide), streaming Wv^T
        for tt in range(4):
            ps_v = psA.tile([128, HL * DK], F32, name="ps_v", tag="pa")
            tcol = tt * 128
            for dj in range(ND):
                nc.tensor.matmul(
                    ps_v,
                    lhsT=xt_sb[:, dj, tcol : tcol + 128],
                    rhs=wv_sb[:, dj, :],
                    start=(dj == 0), stop=(dj == ND - 1),
                )
            kt = tsb * 4 + tt
            for h in range(HL):
                nc.vector.tensor_copy(
                    Vh[h][:, kt, 0:DK], ps_v[:, h * DK : (h + 1) * DK]
                )

    def emit_normalize(h, psav):
        # out^T = psav / sums; sums sit in ones row 64.
        # (reciprocal_approx_fast is a custom DVE op and must read SBUF,
        # not PSUM — feeding it psav directly returns garbage.)
        pa = psav[h]
        sums_sb = spool.tile([1, 512], F32, name="sums_sb")
        nc.vector.tensor_copy(sums_sb, pa[DK : DK + 1, :])
        rt = spool.tile([1, 512], F32, name="rt")
        nc.vector.reciprocal_approx_fast(rt, sums_sb)
        rb = spool.tile([DK, 512], F32, name="rb")
        nc.gpsimd.partition_broadcast(rb, rt, channels=DK)
        if h == 0:
            nc.vector.tensor_mul(ot01[0:DK, :], pa[0:DK, :], rb)
        elif h == 1:
            ot1s = spool.tile([DK, 512], BF16, name="ot1s")
            nc.vector.tensor_mul(ot1s, pa[0:DK, :], rb)
            nc.sync.dma_start(out=ot01[DK:128, :], in_=ot1s)
        else:
            nc.vector.tensor_mul(ot2, pa[0:DK, :], rb)

    def emit_attn_pass(qb, hpass, heads, psav):
        nk = 4 * (qb + 1)

        def emit_av(kts, pts):
            for h in heads:
                for i, kt in enumerate(kts):
                    off = slice(i * 512, (i + 1) * 512)
                    nc.tensor.matmul(
                        psav[h],
                        lhsT=Vh[h][:, kt, :], rhs=pts[h][:, off],
                        start=(kt == 0), stop=(kt == nk - 1),
                    )

        qblk = slice(qb * 512, (qb + 1) * 512)
        pend = None
        for kp in range(nk // 2):
            kt0, kt1 = 2 * kp, 2 * kp + 1
            ss = {h: psA.tile([128, 1024], F32, name=f"ss{h}", tag="pa")
                  for h in heads}
            for i, kt in enumerate((kt0, kt1)):
                kblk = slice(kt * 128, (kt + 1) * 128)
                off = slice(i * 512, (i + 1) * 512)
                if hpass == 0:
                    nc.tensor.matmul(
                        ss[0][:, off], lhsT=KA[0:64, kblk],
                        rhs=QB[0:64, qblk], start=True, stop=True,
                    )
                    nc.tensor.matmul(
                        ss[1][:, off], lhsT=KA[64:128, kblk],
                        rhs=QB[64:128, qblk], start=True, stop=True,
                    )
                elif i == 0:
                    nc.tensor.matmul(
                        ss[2][:, off], lhsT=C2[0:64, kblk],
                        rhs=D2[0:64, qblk], start=True, stop=True,
                    )
                else:
                    nc.tensor.matmul(
                        ss[2][:, off], lhsT=D2[64:128, kblk],
                        rhs=C2[64:128, qblk], start=True, stop=True,
                    )
            pts = {}
            for h in heads:
                if hpass == 1 and kp % 2 == 1:
                    # DVE bitcast-exp: int16 holding the bits of bf16 2^u
                    pi = ptpool.tile([128, 1024], I16, name="pt")
                    nc.vector.tensor_scalar(
                        pi, ss[h], EXP_C1, EXP_C2, ALU.mult, ALU.add
                    )
                    pt = pi.bitcast(BF16)
                else:
                    pt = ptpool.tile([128, 1024], BF16, name="pt")
                    nc.scalar.activation(pt, ss[h], AF.Exp, scale=0.125)
                for i, kt in enumerate((kt0, kt1)):
                    if kt >= 4 * qb:  # diagonal band tile
                        off = slice(i * 512, (i + 1) * 512)
                        nc.vector.tensor_mul(
                            pt[:, off], pt[:, off], bandmask[kt - 4 * qb]
                        )
                pts[h] = pt
            if pend is not None:
                emit_av(*pend)
            pend = ((kt0, kt1), pts)
        emit_av(*pend)

    def emit_outproj(qb, part, psy_tiles, ybig):
        # y^T[d, q]: heads 0/1 stacked on partition halves form one K=128
        # contraction (part 0); head 2's K=64 accumulates on top (part 1).
        # (Mixed ROW positions inside one accumulation group crash the HW,
        # so never pair row-groups within an accumulating chain.)
        qblk = slice(qb * 512, (qb + 1) * 512)
        for dj in range(ND):
            dblk = slice(dj * 128, (dj + 1) * 128)
            if part == 0:
                ps_y = psC.tile([128, 512], F32, name="ps_y", tag="pc")
                nc.tensor.matmul(
                    ps_y, lhsT=wo01_sb[:, dblk], rhs=ot01,
                    start=True, stop=False, skip_group_check=True,
                )
                psy_tiles[dj] = ps_y
            else:
                ps_y = psy_tiles[dj]
                nc.tensor.matmul(
                    ps_y, lhsT=wo2_sb[:, dblk], rhs=ot2,
                    start=False, stop=True, skip_group_check=True,
                )
                nc.vector.tensor_copy(ybig[:, dj, :], ps_y)
        if part == 1:
            nc.sync.dma_start(out=y_v[:, :, qblk], in_=ybig)

    emit_proj(0)
    for qb in range(NQB):
        psav = {}
        psav[0] = psB.tile([DK + 1, 512], F32, name="psav0", tag="pb")
        psav[1] = psB.tile([DK + 1, 512], F32, name="psav1", tag="pb")
        emit_attn_pass(qb, 0, (0, 1), psav)
        emit_normalize(0, psav)
        emit_normalize(1, psav)
        psav[2] = psB.tile([DK + 1, 512], F32, name="psav2", tag="pb")
        emit_attn_pass(qb, 1, (2,), psav)
        emit_normalize(2, psav)
        if qb + 1 < NQB:
            emit_proj(qb + 1)
        psy_tiles = {}
        ybig = ypool.tile([128, ND, 512], BF16, name="ybig")
        emit_outproj(qb, 0, psy_tiles, ybig)
        emit_outproj(qb, 1, psy_tiles, ybig)
    ctx.close()


def build():
    if "nc" in _CACHE:
        return _CACHE["nc"]
    nc = bacc.Bacc(
        "TRN2", target_bir_lowering=False, debug=False, num_devices=NCORES
    )
    with tile.TileContext(nc) as tc:
        _emit(tc)
    nc.compile()
    _CACHE["nc"] = nc
    return nc


def make_in_maps(x, w_qkv, w_out):
    x = np.asarray(x, dtype=np.float32)
    w_qkv = np.asarray(w_qkv, dtype=np.float32)
    w_out = np.asarray(w_out, dtype=np.float32)
    wq = w_qkv[0:D]        # [768, 768], rows = q features
    wk = w_qkv[D : 2 * D]
    wv = w_qkv[2 * D :]
    xT = [np.ascontiguousarray(x[b].T).astype(BF) for b in range(B)]
    in_maps = []
    for c in range(NCORES):
        b, g = divmod(c, 4)
        hs = [3 * g + j for j in range(HL)]  # global head ids
        h0, h1, h2 = hs
        cols = []
        for pair in ((wk, h0), (wk, h1), (wq, h0), (wq, h1), (wk, h2), (wq, h2)):
            w, h = pair
            cols.append(w[h * DK : (h + 1) * DK].T)  # [768, 64]
        wqkT = np.ascontiguousarray(np.concatenate(cols, axis=1))  # [768, 384]
        wvT = np.ascontiguousarray(
            np.concatenate([wv[h * DK : (h + 1) * DK].T for h in hs], axis=1)
        )  # [768, 192]
        woT = np.ascontiguousarray(
            np.stack([w_out[:, h * DK : (h + 1) * DK].T for h in hs])
        )  # [3, 64, 768]
        in_maps.append(
            {
                "xT": xT[b],
                "wqkT": wqkT.astype(BF),
                "wvT": wvT.astype(BF),
                "woT": woT.astype(BF),
            }
        )
    return in_maps


def run(inputs, trace=False):
    """Run on hardware; returns (y [B,T,D] fp32, BassKernelResults)."""
    nc = build()
    in_maps = make_in_maps(inputs["x"], inputs["w_qkv"], inputs["w_out"])
    br = run_bass_kernel_spmd(nc, in_maps, list(range(NCORES)), trace=trace)
    y = np.zeros((B, T, D), dtype=np.float32)
    for c in range(NCORES):
        b = c // 4
        y[b] += np.asarray(br.results[c]["yT"]).astype(np.float32).T
    return y, br


def kernel(x, w_qkv, w_out):
    y, _ = run({"x": x, "w_qkv": w_qkv, "w_out": w_out})
    return y


# revision 16
# speedup vs baseline: 3.1322x; 1.0620x over previous
"""Multi-head causal self-attention (B=2, T=4096, D=768, H=12) on 8 trn2 cores.

Sharding: core c -> batch b = c//4, heads 3*(c%4) .. 3*(c%4)+2.
qkv_proj column-parallel (each core computes Q/K/V only for its heads),
out_proj row-parallel (each core emits a partial y^T; host sums the 4
partials per batch).

Device dataflow (bf16 operands, fp32 PSUM accumulation):
  x^T pre-transposed and cast to bf16 on the host -> Q^T/K^T via
  transposed projection (W^T stationary 128-wide, x^T streaming) ->
  S^T = K Q^T in [k,q] layout, two heads row-paired on opposite PE
  halves -> exp straight out of PSUM into bf16 SBUF; heads 0/1 use
  ScalarE Exp, head 2 alternates ScalarE with a DVE integer-bitcast
  2^u approximation (one tensor_scalar into int16 whose bits are the
  bf16 exp; +-3% sawtooth cancels in the softmax normalization) ->
  causal band masks on DVE (4x-rate all-SBUF bf16) -> out^T = V^T P^T
  with a ones column appended to V collecting softmax denominators ->
  normalize via approx reciprocal + gpsimd partition broadcast ->
  y^T partial = Wo^T.T out^T with heads 0/1 row-paired, stored bf16.

Emission order per step: attention hpass0 (h0/h1) -> normalize h0/h1
-> hpass1 (h2) -> normalize h2 -> projection for the NEXT superblock
-> out-proj of this q-block. The PE queue therefore always has
projection work while the normalize chain runs, and AV matmuls lag one
k-pair behind scores so the PE never head-of-line blocks on exp."""

import sys

sys.path.insert(0, "/opt/trn_rl_repo")

import numpy as np
import ml_dtypes
from contextlib import ExitStack

import concourse.bass as bass
import concourse.bacc as bacc
import concourse.tile as tile
import concourse.mybir as mybir
from concourse.bass_utils import run_bass_kernel_spmd

F32 = mybir.dt.float32
BF16 = mybir.dt.bfloat16
I16 = mybir.dt.int16
AF = mybir.ActivationFunctionType
ALU = mybir.AluOpType
BF = ml_dtypes.bfloat16

B = 2
T = 4096
D = 768
H = 12
DK = 64
NCORES = 8
HL = 3  # heads per core
ND = D // 128  # 6 d-tiles
NKT = T // 128  # 32 k-tiles
NQB = T // 512  # 8 q-blocks
NTSB = T // 512  # 8 t-superblocks (4 t-tiles each)

# DVE bitcast-exp constants: bf16bits(exp(s)) ~ round(s*0.125*log2e*128
# + (16256 - 5.5)); the -5.5 centers the linear-mantissa sawtooth.
EXP_C1 = 0.125 * 1.4426950408889634 * 128.0
EXP_C2 = 16256.0 - 5.5

_CACHE = {}


def _emit(tc):
    nc = tc.nc
    xt_d = nc.dram_tensor("xT", [D, T], BF16, kind="ExternalInput").ap()
    wqk_d = nc.dram_tensor("wqkT", [D, 384], BF16, kind="ExternalInput").ap()
    wv_d = nc.dram_tensor("wvT", [D, HL * DK], BF16, kind="ExternalInput").ap()
    wo_d = nc.dram_tensor("woT", [HL, DK, D], BF16, kind="ExternalInput").ap()
    y_d = nc.dram_tensor("yT", [D, T], BF16, kind="ExternalOutput").ap()
    xt_v = xt_d.rearrange("(j p) t -> p j t", p=128)
    y_v = y_d.rearrange("(j p) t -> p j t", p=128)

    ctx = ExitStack()
    const = ctx.enter_context(tc.tile_pool(name="const", bufs=1))
    persist = ctx.enter_context(tc.tile_pool(name="persist", bufs=1))
    xtpool = ctx.enter_context(tc.tile_pool(name="xt", bufs=2))
    ptpool = ctx.enter_context(tc.tile_pool(name="pt", bufs=5))
    spool = ctx.enter_context(tc.tile_pool(name="sp", bufs=2))
    ypool = ctx.enter_context(tc.tile_pool(name="yp", bufs=2))
    # PSUM (8 banks): pa = streaming (S tiles, qkv, V) 2 bufs x 2 banks;
    # pb = AV accumulators, 2 bufs (h2 reuses h0's slot after normalize
    # h0); pc = yT out-proj, 2 bufs so dj pipelines across the copy.
    psA = ctx.enter_context(tc.tile_pool(name="psA", bufs=2, space="PSUM"))
    psB = ctx.enter_context(tc.tile_pool(name="psB", bufs=2, space="PSUM"))
    psC = ctx.enter_context(tc.tile_pool(name="psC", bufs=2, space="PSUM"))

    # ---- constants ----
    # causal band masks for the 4 diagonal-band k-tiles of each q-block:
    # bandmask[bp][k, q] = 0 for q < 128*bp + k, else 1
    bandmask = []
    for bp in range(4):
        m = const.tile([128, 512], BF16, name=f"bandmask{bp}")
        nc.gpsimd.memset(m, 1.0)
        nc.gpsimd.affine_select(
            out=m, in_=m, compare_op=mybir.AluOpType.is_ge, fill=0.0,
            base=-128 * bp, pattern=[[1, 512]], channel_multiplier=-1,
        )
        bandmask.append(m)

    wqk_sb = const.tile([128, ND, 384], BF16)
    nc.sync.dma_start(out=wqk_sb, in_=wqk_d.rearrange("(j p) e -> p j e", p=128))
    wv_sb = const.tile([128, ND, HL * DK], BF16)
    nc.sync.dma_start(out=wv_sb, in_=wv_d.rearrange("(j p) e -> p j e", p=128))
    wo01_sb = const.tile([128, D], BF16)  # head0 rows on 0:64, head1 on 64:128
    nc.sync.dma_start(out=wo01_sb, in_=wo_d[0:2].rearrange("h p d -> (h p) d"))
    wo2_sb = const.tile([DK, D], BF16)
    nc.sync.dma_start(out=wo2_sb, in_=wo_d[2])

    # ---- persistent activations ----
    # KA: [K^T_h0 ; K^T_h1], QB: [Q^T_h0 ; Q^T_h1] on partition halves
    KA = persist.tile([128, T], BF16, name="KA")
    QB = persist.tile([128, T], BF16, name="QB")
    C2 = persist.tile([128, T], BF16, name="C2")  # [K^T_h2 ; Q^T_h2]
    D2 = persist.tile([128, T], BF16, name="D2")  # [Q^T_h2 ; K^T_h2] (swapped)
    # V natural layout per head + a ones column collecting softmax sums
    Vh = []
    for h in range(HL):
        vt = persist.tile([128, NKT, DK + 1], BF16, name=f"V{h}")
        nc.gpsimd.memset(vt[:, :, DK : DK + 1], 1.0)
        Vh.append(vt)
    ot01 = persist.tile([128, 512], BF16, name="ot01")  # heads 0/1 out^T per qb
    ot2 = persist.tile([DK, 512], BF16, name="ot2")

    qk_dest = [KA, QB, C2]

    def proj_chunks(tsb):
        """Emit the x^T DMA now; return per-(e|t)-tile projection closures
        for interleaved emission."""
        blk = slice(tsb * 512, (tsb + 1) * 512)
        xt_sb = xtpool.tile([128, ND, 512], BF16, name="xt_sb")
        nc.sync.dma_start(out=xt_sb, in_=xt_v[:, :, blk])

        def qk_chunk(et):
            # Q^T / K^T projection: out[e, t] block per e-tile (full 128-wide
            # stationary: e-tile 0 = [K_h0|K_h1], 1 = [Q_h0|Q_h1], 2 = [K_h2|Q_h2])
            def emit():
                ps_q = psA.tile([128, 512], F32, name="ps_q", tag="pa")
                e0 = et * 128
                for dj in range(ND):
                    nc.tensor.matmul(
                        ps_q,
                        lhsT=wqk_sb[:, dj, e0 : e0 + 128],
                        rhs=xt_sb[:, dj, :],
                        start=(dj == 0), stop=(dj == ND - 1),
                    )
                nc.vector.tensor_copy(qk_dest[et][:, blk], ps_q)
                if et == 2:
                    # D2 = partition-swapped copy of C2 (h2 self-pairing)
                    nc.sync.dma_start(out=D2[0:64, blk], in_=C2[64:128, blk])
                    nc.sync.dma_start(out=D2[64:128, blk], in_=C2[0:64, blk])
            return emit

        def v_chunk(tt):
            # V natural: stationary x^T tiles (full 128-wide), streaming Wv^T
            def emit():
                ps_v = psA.tile([128, HL * DK], F32, name="ps_v", tag="pa")
                tcol = tt * 128
                for dj in range(ND):
                    nc.tensor.matmul(
                        ps_v,
                        lhsT=xt_sb[:, dj, tcol : tcol + 128],
                        rhs=wv_sb[:, dj, :],
                        start=(dj == 0), stop=(dj == ND - 1),
                    )
                kt = tsb * 4 + tt
                for h in range(HL):
                    nc.vector.tensor_copy(
                        Vh[h][:, kt, 0:DK], ps_v[:, h * DK : (h + 1) * DK]
                    )
            return emit

        return [qk_chunk(et) for et in range(3)] + [v_chunk(tt) for tt in range(4)]

    def emit_proj(tsb):
        for c in proj_chunks(tsb):
            c()

    def emit_normalize(h, psav):
        # out^T = psav / sums; sums sit in ones row 64.
        # (reciprocal_approx_fast is a custom DVE op and must read SBUF,
        # not PSUM — feeding it psav directly returns garbage.)
        pa = psav[h]
        sums_sb = spool.tile([1, 512], F32, name="sums_sb")
        nc.vector.tensor_copy(sums_sb, pa[DK : DK + 1, :])
        rt = spool.tile([1, 512], F32, name="rt")
        nc.vector.reciprocal_approx_fast(rt, sums_sb)
        rb = spool.tile([DK, 512], F32, name="rb")
        nc.gpsimd.partition_broadcast(rb, rt, channels=DK)
        if h == 0:
            nc.vector.tensor_mul(ot01[0:DK, :], pa[0:DK, :], rb)
        elif h == 1:
            ot1s = spool.tile([DK, 512], BF16, name="ot1s")
            nc.vector.tensor_mul(ot1s, pa[0:DK, :], rb)
            nc.sync.dma_start(out=ot01[DK:128, :], in_=ot1s)
        else:
            nc.vector.tensor_mul(ot2, pa[0:DK, :], rb)

    def emit_attn_pass(qb, hpass, heads, psav, chunks=()):
        # chunks: deferred emission closures (projection pieces / out-proj)
        # interleaved one-per-kp so the PE queue stays fed while ScalarE
        # paces the exp stream.
        nk = 4 * (qb + 1)
        chunks = list(chunks)

        def emit_av(kts, pts):
            for h in heads:
                for i, kt in enumerate(kts):
                    # band tiles attend only to q >= 128*bp within the block
                    lo = (kt - 4 * qb) * 128 if kt >= 4 * qb else 0
                    off = slice(i * 512 + lo, (i + 1) * 512)
                    nc.tensor.matmul(
                        psav[h][:, lo:512],
                        lhsT=Vh[h][:, kt, :], rhs=pts[h][:, off],
                        start=(kt == 0), stop=(kt == nk - 1),
                    )

        qblk = qb * 512
        pend = None
        for kp in range(nk // 2):
            kt0, kt1 = 2 * kp, 2 * kp + 1
            ss = {h: psA.tile([128, 1024], F32, name=f"ss{h}", tag="pa")
                  for h in heads}
            for i, kt in enumerate((kt0, kt1)):
                kblk = slice(kt * 128, (kt + 1) * 128)
                lo = (kt - 4 * qb) * 128 if kt >= 4 * qb else 0
                off = slice(i * 512 + lo, (i + 1) * 512)
                qrng = slice(qblk + lo, qblk + 512)
                if hpass == 0:
                    nc.tensor.matmul(
                        ss[0][:, off], lhsT=KA[0:64, kblk],
                        rhs=QB[0:64, qrng], start=True, stop=True,
                    )
                    nc.tensor.matmul(
                        ss[1][:, off], lhsT=KA[64:128, kblk],
                        rhs=QB[64:128, qrng], start=True, stop=True,
                    )
                elif i == 0:
                    nc.tensor.matmul(
                        ss[2][:, off], lhsT=C2[0:64, kblk],
                        rhs=D2[0:64, qrng], start=True, stop=True,
                    )
                else:
                    nc.tensor.matmul(
                        ss[2][:, off], lhsT=D2[64:128, kblk],
                        rhs=C2[64:128, qrng], start=True, stop=True,
                    )
            pts = {}
            for h in heads:
                if hpass == 0 and h == 1 and kp % 2 == 1:
                    # DVE bitcast-exp: int16 holding the bits of bf16 2^u.
                    # (exp of stale PSUM in the masked band strips is
                    # harmless: stale values are bounded scores/projections.)
                    pi = ptpool.tile([128, 1024], I16, name="pt")
                    nc.vector.tensor_scalar(
                        pi, ss[h], EXP_C1, EXP_C2, ALU.mult, ALU.add
                    )
                    pt = pi.bitcast(BF16)
                else:
                    pt = ptpool.tile([128, 1024], BF16, name="pt")
                    nc.scalar.activation(pt, ss[h], AF.Exp, scale=0.125)
                for i, kt in enumerate((kt0, kt1)):
                    if kt >= 4 * qb:  # diagonal band tile
                        off = slice(i * 512, (i + 1) * 512)
                        nc.vector.tensor_mul(
                            pt[:, off], pt[:, off], bandmask[kt - 4 * qb]
                        )
                pts[h] = pt
            if pend is not None:
                emit_av(*pend)
            pend = ((kt0, kt1), pts)
            if chunks:
                chunks.pop(0)()
        emit_av(*pend)
        for c in chunks:
            c()

    def emit_outproj(qb, part, psy_tiles, ybig):
        # y^T[d, q]: heads 0/1 stacked on partition halves form one K=128
        # contraction (part 0); head 2's K=64 accumulates on top (part 1).
        # (Mixed ROW positions inside one accumulation group crash the HW,
        # so never pair row-groups within an accumulating chain.)
        qblk = slice(qb * 512, (qb + 1) * 512)
        for dj in range(ND):
            dblk = slice(dj * 128, (dj + 1) * 128)
            if part == 0:
                ps_y = psC.tile([128, 512], F32, name="ps_y", tag="pc")
                nc.tensor.matmul(
                    ps_y, lhsT=wo01_sb[:, dblk], rhs=ot01,
                    start=True, stop=False, skip_group_check=True,
                )
                psy_tiles[dj] = ps_y
            else:
                ps_y = psy_tiles[dj]
                nc.tensor.matmul(
                    ps_y, lhsT=wo2_sb[:, dblk], rhs=ot2,
                    start=False, stop=True, skip_group_check=True,
                )
                nc.vector.tensor_copy(ybig[:, dj, :], ps_y)
        if part == 1:
            nc.sync.dma_start(out=y_v[:, :, qblk], in_=ybig)

    emit_proj(0)
    for qb in range(NQB):
        psav = {}
        psav[0] = psB.tile([DK + 1, 512], F32, name="psav0", tag="pb")
        psav[1] = psB.tile([DK + 1, 512], F32, name="psav1", tag="pb")
        emit_attn_pass(qb, 0, (0, 1), psav)
        emit_normalize(0, psav)
        emit_normalize(1, psav)
        psav[2] = psB.tile([DK + 1, 512], F32, name="psav2", tag="pb")
        psy_tiles = {}
        ybig = ypool.tile([128, ND, 512], BF16, name="ybig")
        if qb + 1 < NQB:
            chunks = proj_chunks(qb + 1)
        else:
            chunks = [lambda: emit_outproj(qb, 0, psy_tiles, ybig)]
        emit_attn_pass(qb, 1, (2,), psav, chunks)
        emit_normalize(2, psav)
        if qb + 1 < NQB:
            emit_outproj(qb, 0, psy_tiles, ybig)
        emit_outproj(qb, 1, psy_tiles, ybig)
    ctx.close()


def build():
    if "nc" in _CACHE:
        return _CACHE["nc"]
    nc = bacc.Bacc(
        "TRN2", target_bir_lowering=False, debug=False, num_devices=NCORES
    )
    with tile.TileContext(nc) as tc:
        _emit(tc)
    nc.compile()
    _CACHE["nc"] = nc
    return nc


def make_in_maps(x, w_qkv, w_out):
    x = np.asarray(x, dtype=np.float32)
    w_qkv = np.asarray(w_qkv, dtype=np.float32)
    w_out = np.asarray(w_out, dtype=np.float32)
    wq = w_qkv[0:D]        # [768, 768], rows = q features
    wk = w_qkv[D : 2 * D]
    wv = w_qkv[2 * D :]
    xT = [np.ascontiguousarray(x[b].T).astype(BF) for b in range(B)]
    in_maps = []
    for c in range(NCORES):
        b, g = divmod(c, 4)
        hs = [3 * g + j for j in range(HL)]  # global head ids
        h0, h1, h2 = hs
        cols = []
        for pair in ((wk, h0), (wk, h1), (wq, h0), (wq, h1), (wk, h2), (wq, h2)):
            w, h = pair
            cols.append(w[h * DK : (h + 1) * DK].T)  # [768, 64]
        wqkT = np.ascontiguousarray(np.concatenate(cols, axis=1))  # [768, 384]
        wvT = np.ascontiguousarray(
            np.concatenate([wv[h * DK : (h + 1) * DK].T for h in hs], axis=1)
        )  # [768, 192]
        woT = np.ascontiguousarray(
            np.stack([w_out[:, h * DK : (h + 1) * DK].T for h in hs])
        )  # [3, 64, 768]
        in_maps.append(
            {
                "xT": xT[b],
                "wqkT": wqkT.astype(BF),
                "wvT": wvT.astype(BF),
                "woT": woT.astype(BF),
            }
        )
    return in_maps


def run(inputs, trace=False):
    """Run on hardware; returns (y [B,T,D] fp32, BassKernelResults)."""
    nc = build()
    in_maps = make_in_maps(inputs["x"], inputs["w_qkv"], inputs["w_out"])
    br = run_bass_kernel_spmd(nc, in_maps, list(range(NCORES)), trace=trace)
    y = np.zeros((B, T, D), dtype=np.float32)
    for c in range(NCORES):
        b = c // 4
        y[b] += np.asarray(br.results[c]["yT"]).astype(np.float32).T
    return y, br


def kernel(x, w_qkv, w_out):
    y, _ = run({"x": x, "w_qkv": w_qkv, "w_out": w_out})
    return y
